# revision 1
# baseline (speedup 1.0000x reference)
"""Trainium2 Bass kernel for single-head attention (no mask).

Reference computation (B=4, S=2048, D=1024):
    q = x @ Wq.T ; k = x @ Wk.T ; v = x @ Wv.T          (per batch)
    out = softmax((q @ k.T) / sqrt(1024)) @ v

Sharding: 8 cores = (batch, query-half); each core computes its 1024
query rows against the full 2048-key sequence of its batch (attention
is invariant to the common row permutation that puts the core's query
half first). No collectives (a pair K/V exchange costs ~225us in the
measured-collective model vs ~55us of PE it would save).

Algebraic structure (keys/values never materialized):
    scores   = q k^T = x (Wq^T Wk) x^T      with M = Wq^T Wk from host
    out      = softmax(scores) (x Wv^T) = (softmax(scores) x) Wv^T
so the device work is four dense stages, all operands bf16 in SBUF:
    A: H   = M^T x_q^T                 [d,  q]   65.5k PE cycles
    B: S^T = x H   -> exp -> E^T       [k,  q]  131k (+16.4k rowsum)
    C: C^T = x^T E^T  (/rowsum)        [d,  q]  131k
    D: o^T = Wv C^T                    [o,  q]   65.5k
vs ~648k cycles for the direct QKV formulation: projections shrink to
the 1024-wide post-softmax contraction, M/Wv^T/x^T/x are host-prepped.

The emission order keeps the PE stream gap-free (the cost model resets
the clock-ramp p-state on every idle gap): a warmup matmul chain holds
the PE from t~0.4us until the first loads land, phase A runs 8 chains
interleaved (borrowing the 2 rowsum PSUM banks) so consumption stays
behind the single-queue DMA feed, chain finishes are staggered with
evictions split across DVE+Act, exp/rowsum interleave one key-tile
behind the scores, and the final rowsum + reciprocal hide inside phase
C's first chain.
"""

import ml_dtypes
import numpy as np

import concourse.bass as bass
import concourse.tile as tile
from concourse import bacc, bass_isa, mybir
from concourse.bass_utils import run_bass_kernel_spmd

B, S, D, O = 4, 2048, 1024, 1024
HQ = S // 2  # query rows per core
N_CORES = 8
BF = mybir.dt.bfloat16
F32 = mybir.dt.float32
SCALE = 1.0 / 32.0  # 1/sqrt(1024)
DK = D // 128  # 8 contraction tiles over d
KT = S // 128  # 16 key tiles
OT = O // 128  # 8 output o-tiles
NWARM = 31  # warmup matmuls bridging the initial DMA latency

_CACHE: dict = {}


def _emit(nc, sfx=""):
    xt_d = nc.dram_tensor(f"xt{sfx}", [D, S], BF, kind="ExternalInput")
    xn_d = nc.dram_tensor(f"xn{sfx}", [S, D], BF, kind="ExternalInput")
    m_d = nc.dram_tensor(f"m{sfx}", [D, D], BF, kind="ExternalInput")
    wvt_d = nc.dram_tensor(f"wvt{sfx}", [D, O], BF, kind="ExternalInput")
    out_d = nc.dram_tensor(f"outT{sfx}", [O, HQ], F32, kind="ExternalOutput")

    with tile.TileContext(nc) as tc:
        with (
            tc.tile_pool(name=f"{sfx}sb", bufs=1) as sb,
            tc.tile_pool(name=f"{sfx}pp", bufs=6, space="PSUM") as pp,
            tc.tile_pool(name=f"{sfx}rs", bufs=2, space="PSUM") as rs,
        ):
            # Per-block tiles so dependency tracking stays fine-grained.
            mt = [sb.tile([128, D], BF, tag=f"mt{i}", name=f"mt{sfx}_{i}") for i in range(DK)]
            xq = [sb.tile([128, HQ], BF, tag=f"xq{i}", name=f"xq{sfx}_{i}") for i in range(DK)]
            xk = [sb.tile([128, HQ], BF, tag=f"xk{i}", name=f"xk{sfx}_{i}") for i in range(DK)]
            xn = [sb.tile([128, D], BF, tag=f"xn{i}", name=f"xn{sfx}_{i}") for i in range(KT)]
            wv = [sb.tile([128, O], BF, tag=f"wv{i}", name=f"wv{sfx}_{i}") for i in range(DK)]
            ht = [sb.tile([128, HQ], BF, tag=f"ht{i}", name=f"ht{sfx}_{i}") for i in range(DK)]
            et = [sb.tile([128, HQ], BF, tag=f"et{i}", name=f"et{sfx}_{i}") for i in range(KT)]
            ct = [sb.tile([128, HQ], BF, tag=f"ct{i}", name=f"ct{sfx}_{i}") for i in range(DK)]
            onest = sb.tile([128, 128], BF, tag="ones", name=f"onest{sfx}")
            warmt = sb.tile([128, 128], BF, tag="warm", name=f"warmt{sfx}")
            recip = sb.tile([128, HQ], F32, tag="recip", name=f"recip{sfx}")
            rowacc = sb.tile([128, HQ], F32, tag="rowacc", name=f"rowacc{sfx}")

            # Constants via memset (no DMA bandwidth). warmt on DVE so
            # the warmup chain can start early.
            nc.vector.memset(warmt, 0.0)
            nc.gpsimd.memset(onest, 1.0)
            # Dummy exp: hoists the Act engine's one-time activation-table
            # load into the idle startup window (it otherwise lands right
            # before the phase-A hi-evictions and delays them).
            actwarm = sb.tile([128, 1], BF, tag="actwarm", name=f"actwarm{sfx}")
            nc.scalar.activation(
                out=actwarm,
                in_=onest[:, 0:1],
                func=mybir.ActivationFunctionType.Exp,
                scale=SCALE,
            )

            # ---- DMA loads: one in-order queue = explicit priority ----
            # (mtl_i, xq_i, mth_i) triplets feed phase A; everything later
            # is needed tens of us after it lands.
            for i in range(DK):
                nc.sync.dma_start(out=mt[i], in_=m_d[i * 128 : (i + 1) * 128, :])
                nc.sync.dma_start(out=xq[i], in_=xt_d[i * 128 : (i + 1) * 128, 0:HQ])
            for i in range(DK):
                nc.sync.dma_start(out=xk[i], in_=xt_d[i * 128 : (i + 1) * 128, HQ:S])
            for i in range(DK):
                nc.sync.dma_start(out=wv[i], in_=wvt_d[i * 128 : (i + 1) * 128, :])
            for i in range(KT):
                nc.sync.dma_start(out=xn[i], in_=xn_d[i * 128 : (i + 1) * 128, :])

            # ---- PE warmup: hold the p-state until the first loads land ----
            wps = pp.tile([128, 512], F32, tag="ps", name=f"wps{sfx}")
            for i in range(NWARM):
                nc.tensor.matmul(wps[:, 0:128], warmt, warmt, start=True, stop=True)

            # ---- Phase A: H = M^T x_q^T ----
            a_ps = {}

            def a_mm(ch, d1t):
                d2t, qc = divmod(ch, 2)
                nc.tensor.matmul(
                    a_ps[ch],
                    mt[d1t][:, d2t * 128 : (d2t + 1) * 128],
                    xq[d1t][:, qc * 512 : (qc + 1) * 512],
                    start=(d1t == 0),
                    stop=(d1t == DK - 1),
                )

            def a_evict(ch):
                # Alternate DVE / Act per chain so eviction keeps pace with
                # the PE and the first eviction lands as early as possible.
                d2t, qc = divmod(ch, 2)
                dst = ht[d2t][:, qc * 512 : (qc + 1) * 512]
                if ch % 2 == 0:
                    nc.vector.tensor_copy(out=dst, in_=a_ps[ch])
                else:
                    nc.scalar.copy(out=dst, in_=a_ps[ch])

            # Two 8-chain interleaved waves: PE consumes one (mtl, xq, mth)
            # DMA triplet per 8 matmuls, slower than the DMA feed; chain
            # finishes are staggered so evictions free banks early.
            for wave in range(2):
                lo = 8 * wave
                for j, ch in enumerate(range(lo, lo + 8)):
                    pool = pp if j < 6 else rs
                    tag = "ps" if j < 6 else "rs"
                    a_ps[ch] = pool.tile(
                        [128, 512], F32, tag=tag, name=f"aps{sfx}_{ch}"
                    )
                for d1t in range(DK - 1):
                    for ch in range(lo, lo + 8):
                        a_mm(ch, d1t)
                for ch in range(lo, lo + 8):
                    a_mm(ch, DK - 1)
                    a_evict(ch)

            # ---- Phase B: S^T = x H, exp, rowsums ----
            # Rowsums run entirely off-PE: Pool partition-reduces each
            # exp'd key tile, DVE accumulates across tiles.
            def rowsum(kt):
                for qc in range(2):
                    rtmp = sb.tile(
                        [128, 512], F32, tag=f"rtmp{qc}", bufs=2,
                        name=f"rtmp{sfx}_{kt}_{qc}",
                    )
                    nc.gpsimd.partition_all_reduce(
                        rtmp,
                        et[kt][:, qc * 512 : (qc + 1) * 512],
                        128,
                        bass_isa.ReduceOp.add,
                    )
                    if kt == 0:
                        nc.vector.tensor_copy(
                            out=rowacc[:, qc * 512 : (qc + 1) * 512], in_=rtmp
                        )
                    else:
                        nc.vector.scalar_tensor_tensor(
                            out=rowacc[:, qc * 512 : (qc + 1) * 512],
                            in0=rtmp,
                            scalar=0.0,
                            in1=rowacc[:, qc * 512 : (qc + 1) * 512],
                            op0=mybir.AluOpType.bypass,
                            op1=mybir.AluOpType.add,
                        )

            for kt in range(KT):
                xsrc, ki = (xq, kt) if kt < DK else (xk, kt - DK)
                for qc in range(2):
                    sp = pp.tile([128, 512], F32, tag="ps", name=f"sps{sfx}_{kt}_{qc}")
                    for d2t in range(DK):
                        nc.tensor.matmul(
                            sp,
                            xsrc[d2t][:, ki * 128 : (ki + 1) * 128],
                            ht[d2t][:, qc * 512 : (qc + 1) * 512],
                            start=(d2t == 0),
                            stop=(d2t == DK - 1),
                        )
                    nc.scalar.activation(
                        out=et[kt][:, qc * 512 : (qc + 1) * 512],
                        in_=sp,
                        func=mybir.ActivationFunctionType.Exp,
                        scale=SCALE,
                    )
                rowsum(kt)

            # ---- Phase C: C^T = x^T E^T, normalized at eviction ----
            for ch in range(16):
                dt, qc = divmod(ch, 2)
                c_ps = pp.tile([128, 512], F32, tag="ps", name=f"cps{sfx}_{ch}")
                for kt in range(KT):
                    nc.tensor.matmul(
                        c_ps,
                        xn[kt][:, dt * 128 : (dt + 1) * 128],
                        et[kt][:, qc * 512 : (qc + 1) * 512],
                        start=(kt == 0),
                        stop=(kt == KT - 1),
                    )
                if ch == 0:
                    for qc2 in range(2):
                        nc.vector.reciprocal(
                            out=recip[:, qc2 * 512 : (qc2 + 1) * 512],
                            in_=rowacc[:, qc2 * 512 : (qc2 + 1) * 512],
                        )
                nc.vector.scalar_tensor_tensor(
                    out=ct[dt][:, qc * 512 : (qc + 1) * 512],
                    in0=c_ps,
                    scalar=0.0,
                    in1=recip[:, qc * 512 : (qc + 1) * 512],
                    op0=mybir.AluOpType.bypass,
                    op1=mybir.AluOpType.mult,
                )

            # ---- Phase D: out^T = Wv C^T ----
            for ch in range(15):
                ot, qc = divmod(ch, 2)
                d_ps = pp.tile([128, 512], F32, tag="ps", name=f"dps{sfx}_{ch}")
                for dk in range(DK):
                    nc.tensor.matmul(
                        d_ps,
                        wv[dk][:, ot * 128 : (ot + 1) * 128],
                        ct[dk][:, qc * 512 : (qc + 1) * 512],
                        start=(dk == 0),
                        stop=(dk == DK - 1),
                    )
                oev = sb.tile(
                    [128, 512], F32, tag="oev", bufs=3, name=f"oev{sfx}_{ch}"
                )
                nc.vector.tensor_copy(out=oev, in_=d_ps)
                dma_eng = nc.sync if ch % 2 == 0 else nc.gpsimd
                dma_eng.dma_start(
                    out=out_d[ot * 128 : (ot + 1) * 128, qc * 512 : (qc + 1) * 512],
                    in_=oev,
                )
            # Final (ot7, qc1) chain as 4 [128,128] sub-chains so the tail
            # is one narrow evict+DMA instead of a full 512-wide one.
            for c4 in range(4):
                f_ps = pp.tile([128, 512], F32, tag="ps", name=f"fps{sfx}_{c4}")[
                    :, 0:128
                ]
                lo = 512 + c4 * 128
                for dk in range(DK):
                    nc.tensor.matmul(
                        f_ps,
                        wv[dk][:, 7 * 128 : 8 * 128],
                        ct[dk][:, lo : lo + 128],
                        start=(dk == 0),
                        stop=(dk == DK - 1),
                    )
                fev = sb.tile([128, 128], F32, tag="fev", bufs=4, name=f"fev{sfx}_{c4}")
                if c4 % 2 == 0:
                    nc.vector.tensor_copy(out=fev, in_=f_ps)
                else:
                    nc.scalar.copy(out=fev, in_=f_ps)
                dma_eng = nc.gpsimd if c4 % 2 == 0 else nc.sync
                dma_eng.dma_start(
                    out=out_d[7 * 128 : 8 * 128, lo : lo + 128], in_=fev
                )
    return nc


def _get_program():
    if "nc" not in _CACHE:
        nc = bacc.Bacc("TRN2", target_bir_lowering=False, num_devices=N_CORES)
        _emit(nc)
        nc.compile()
        _CACHE["nc"] = nc
    return _CACHE["nc"]


def kernel(x, Wq, Wk, Wv):
    bf = ml_dtypes.bfloat16
    x = np.asarray(x, dtype=np.float32)
    Wq = np.asarray(Wq, dtype=np.float32)
    Wk = np.asarray(Wk, dtype=np.float32)
    Wv = np.asarray(Wv, dtype=np.float32)

    nc = _get_program()
    m = np.ascontiguousarray(Wq.T @ Wk).astype(bf)  # M = Wq^T Wk, [d1, d2]
    wvt = np.ascontiguousarray(Wv.T).astype(bf)  # [D, O]
    in_maps = []
    for c in range(N_CORES):
        b, h = divmod(c, 2)
        xp = np.concatenate(
            [x[b, h * HQ : (h + 1) * HQ], x[b, (1 - h) * HQ : (2 - h) * HQ]], axis=0
        )
        in_maps.append(
            {
                "xt": np.ascontiguousarray(xp.T).astype(bf),
                "xn": xp.astype(bf),
                "m": m,
                "wvt": wvt,
            }
        )
    res = run_bass_kernel_spmd(nc, in_maps, list(range(N_CORES)))
    outp = np.empty((B, S, O), dtype=np.float32)
    for c in range(N_CORES):
        b, h = divmod(c, 2)
        outp[b, h * HQ : (h + 1) * HQ] = res.results[c]["outT"].T
    return outp



# revision 11
# speedup vs baseline: 1.3883x; 1.3883x over previous
"""Trainium2 Bass kernel for single-head attention (no mask), fp8 DoubleRow.

Reference computation (B=4, S=2048, D=1024):
    q = x @ Wq.T ; k = x @ Wk.T ; v = x @ Wv.T          (per batch)
    out = softmax((q @ k.T) / sqrt(1024)) @ v

Sharding: 8 cores = (batch, query-half), same as the bf16 baseline; no
collectives.  Algebra: scores = x (Wq^T Wk) x^T with M = Wq^T Wk
host-prepped, out = softmax(scores) x Wv^T.

All four dense stages run as fp8e4 (e4m3) DoubleRow matmuls: the PE
contracts two 128-row k-tiles per instruction at 0.5 cycles/moving-row,
4x the bf16 MAC rate.  e4m3's ~3.6% quantization noise is managed by
hi+lo residual splitting (a = fp8(a) + fp8(a - fp8(a))) with the number
of product terms chosen per stage, and by a Taylor shift of the softmax:
    E = exp(s) = 1 + Etil,   C = E^T x = colsum(x) (+) Etil^T x
so the rank-1 mass of E (the dominant part) flows through an exact f32
side-channel (colsum via tiny ones-matmuls on the PE, ~free) and only
the small Etil (std ~0.37) is quantized -- cutting its error ~3x.

Stages (per core, q = the core's 1024 queries, 64x scale keeps fp8
operands out of the denormal range):
    A: H = (64 M)^T xq^T        [d,q]  terms m8*x8 + mr8*x8 + m8*r8
    B: S = x H                  [k,q]  terms x8*(h8 + hr8)
    Etil = exp(S/2048) - 1 - fp8 (Act bias), rowsum via Pool reduce
    C: Ct = Etil^T x            [d,q]  term  x8^T e8
    D: o = (64 Wv^T)^T C        [o,q]  terms (wv8+wvr8)*c8 + wv8*cr8
       + v0 = (64 Wv)^T colsum  [o,1]  via tiny [*,1] DoubleRow chains
    evict: out = (d_ps + v0/2) * (2/(64*rowsum))   (one DVE STT)

Error (vs f32 reference, measured): 1.58e-2 mean-rel (gate 2e-2).
PE work: (49.2 + 65.5 + 32.8 + 49.2)k cycles ~= 82 us @ 2.4 GHz vs
393k cycles (164 us) for the bf16 baseline.

Scheduling: one in-order SP DMA queue ordered by first use; a warmup
matmul chain bridges the initial DMA latency; phase-A waves are
qc-major so phase B can chase wave-0's evictions; filler matmuls at
phase junctions keep the PE stream gap-free (the cost model drops to
the mid p-state for 3 us after any idle gap); tiny colsum/v0 chains
interleave into the C/D instruction stream using a reserved PSUM bank;
the final output chain is split into narrow [128,128] sub-chains so the
tail is one small evict+DMA.
"""

import ml_dtypes
import numpy as np

import concourse.bass as bass
import concourse.tile as tile
from concourse import bacc, bass_isa, mybir
from concourse.bass_utils import run_bass_kernel_spmd

B, S, D, O = 4, 2048, 1024, 1024
HQ = S // 2  # query rows per core
N_CORES = 8
BF = mybir.dt.bfloat16
F8 = mybir.dt.float8e4
F32 = mybir.dt.float32
DR = mybir.MatmulPerfMode.DoubleRow
EXP_SCALE = 1.0 / (32.0 * 64.0)  # softmax 1/sqrt(1024) and the 64x M scale
DP = D // 256  # 4 contraction pair-tiles over d
KP = S // 256  # 8 key pair-tiles
NWARM = 56  # warmup matmuls bridging the initial DMA latency

_CACHE: dict = {}


def _emit(nc, sfx=""):
    m8_d = nc.dram_tensor(f"m8{sfx}", [DP, 128, 2, D], F8, kind="ExternalInput")
    mr8_d = nc.dram_tensor(f"mr8{sfx}", [DP, 128, 2, D], F8, kind="ExternalInput")
    x8t_d = nc.dram_tensor(f"x8t{sfx}", [DP, 128, 2, S], F8, kind="ExternalInput")
    r8tq_d = nc.dram_tensor(f"r8tq{sfx}", [DP, 128, 2, HQ], F8, kind="ExternalInput")
    x8n_d = nc.dram_tensor(f"x8n{sfx}", [KP, 128, 2, D], F8, kind="ExternalInput")
    r8n_d = nc.dram_tensor(f"r8n{sfx}", [KP, 128, 2, D], F8, kind="ExternalInput")
    wv8_d = nc.dram_tensor(f"wv8{sfx}", [DP, 128, 2, O], F8, kind="ExternalInput")
    wvr8_d = nc.dram_tensor(f"wvr8{sfx}", [DP, 128, 2, O], F8, kind="ExternalInput")
    out_d = nc.dram_tensor(f"outT{sfx}", [O, HQ], F32, kind="ExternalOutput")

    with tile.TileContext(nc) as tc:
        with (
            tc.tile_pool(name=f"{sfx}sb", bufs=1) as sb,
            tc.tile_pool(name=f"{sfx}pp", bufs=6, space="PSUM") as pp,
            tc.tile_pool(name=f"{sfx}rs", bufs=2, space="PSUM") as rs,
        ):
            m8 = [sb.tile([128, 2, D], F8, tag=f"m8_{j}", name=f"m8{sfx}_{j}") for j in range(DP)]
            mr8 = [sb.tile([128, 2, D], F8, tag=f"mr8_{j}", name=f"mr8{sfx}_{j}") for j in range(DP)]
            x8t = [sb.tile([128, 2, S], F8, tag=f"x8t_{j}", name=f"x8t{sfx}_{j}") for j in range(DP)]
            r8tq = [sb.tile([128, 2, HQ], F8, tag=f"r8tq_{j}", name=f"r8tq{sfx}_{j}") for j in range(DP)]
            x8n = [sb.tile([128, 2, D], F8, tag=f"x8n_{m}", name=f"x8n{sfx}_{m}") for m in range(KP)]
            r8n = [sb.tile([128, 2, D], F8, tag=f"r8n_{m}", name=f"r8n{sfx}_{m}") for m in range(KP)]
            wv8 = [sb.tile([128, 2, O], F8, tag=f"wv8_{j}", name=f"wv8{sfx}_{j}") for j in range(DP)]
            wvr8 = [sb.tile([128, 2, O], F8, tag=f"wvr8_{j}", name=f"wvr8{sfx}_{j}") for j in range(DP)]
            h8 = [sb.tile([128, 2, HQ], F8, tag=f"h8_{j}", name=f"h8{sfx}_{j}") for j in range(DP)]
            hr8 = [sb.tile([128, 2, HQ], F8, tag=f"hr8_{j}", name=f"hr8{sfx}_{j}") for j in range(DP)]
            et8 = [sb.tile([128, 2, HQ], F8, tag=f"et8_{m}", name=f"et8{sfx}_{m}") for m in range(KP)]
            c8 = [sb.tile([128, 2, HQ], F8, tag=f"c8_{j}", name=f"c8{sfx}_{j}") for j in range(DP)]
            cr8 = [sb.tile([128, 2, HQ], F8, tag=f"cr8_{j}", name=f"cr8{sfx}_{j}") for j in range(DP)]
            cs8 = sb.tile([128, 2, DP], F8, tag="cs8", name=f"cs8{sfx}")
            csr8 = sb.tile([128, 2, DP], F8, tag="csr8", name=f"csr8{sfx}")
            ones8 = sb.tile([128, 2, 1], F8, tag="ones8", name=f"ones8{sfx}")
            v0sb = sb.tile([128, 8], F32, tag="v0sb", name=f"v0sb{sfx}")
            warmt = sb.tile([128, 128], BF, tag="warm", name=f"warmt{sfx}")
            rowacc = sb.tile([128, HQ], F32, tag="rowacc", name=f"rowacc{sfx}")
            recip2 = sb.tile([128, HQ], F32, tag="recip2", name=f"recip2{sfx}")

            # Constants via memset (no DMA bandwidth). warmt on DVE so the
            # warmup chain can start early; ones8 (fp8) on Pool.
            nc.vector.memset(warmt, 0.0)
            nc.gpsimd.memset(ones8, 1.0)
            # Dummy exp: hoists the Act engine's one-time activation-table
            # load into the idle startup window.
            actwarm = sb.tile([128, 1], BF, tag="actwarm", name=f"actwarm{sfx}")
            nc.scalar.activation(
                out=actwarm,
                in_=warmt[:, 0:1],
                func=mybir.ActivationFunctionType.Exp,
                scale=EXP_SCALE,
            )

            # ---- DMA loads: one in-order queue = explicit priority ----
            for j in range(DP):
                nc.sync.dma_start(out=m8[j], in_=m8_d[j])
                nc.sync.dma_start(out=x8t[j], in_=x8t_d[j])
            for j in range(DP):
                nc.sync.dma_start(out=mr8[j], in_=mr8_d[j])
            for j in range(DP):
                nc.sync.dma_start(out=r8tq[j], in_=r8tq_d[j])
            for m in range(KP):
                nc.sync.dma_start(out=x8n[m], in_=x8n_d[m])
            for m in range(KP):
                nc.sync.dma_start(out=r8n[m], in_=r8n_d[m])
            for j in range(DP):
                nc.sync.dma_start(out=wv8[j], in_=wv8_d[j])
            for j in range(DP):
                nc.sync.dma_start(out=wvr8[j], in_=wvr8_d[j])

            # ---- PE warmup: hold the p-state until the first loads land ----
            wps = rs.tile([128, 512], F32, tag="rs", name=f"wps{sfx}")
            for i in range(NWARM):
                nc.tensor.matmul(wps[:, 0:128], warmt, warmt, start=True, stop=True)

            fill_n = [0]

            def filler(n):
                # Junction filler: keeps the PE busy (p-state) while vector
                # engines produce the next phase's operands.
                fill_n[0] += 1
                fps = rs.tile([128, 512], F32, tag="rs", name=f"fil{sfx}_{fill_n[0]}")
                for _ in range(n):
                    nc.tensor.matmul(fps[:, 0:128], warmt, warmt, start=True, stop=True)

            # ---- Phase A: H = (64 M)^T xq^T, evicted as fp8 hi+lo ----
            # Wave 0 = qc 0 (so phase B's qc-0 chains can chase wave-0's
            # evictions), wave 1 = qc 1.  Terms j-major so the DMA feed
            # (m8, x8t first, then mr8, then r8tq) is consumed in order.
            for qc in range(2):
                a_ps = {}
                for d1t in range(8):
                    pool = pp if d1t < 6 else rs
                    tag = "ps" if d1t < 6 else "rs"
                    a_ps[d1t] = pool.tile([128, 512], F32, tag=tag, name=f"aps{sfx}_{qc}_{d1t}")
                for term in range(3):
                    lhs, rhs = [(m8, x8t), (mr8, x8t), (m8, r8tq)][term]
                    qlo = qc * 512
                    for j in range(DP):
                        for d1t in range(8):
                            nc.tensor.matmul(
                                a_ps[d1t],
                                lhs[j][:, :, d1t * 128 : (d1t + 1) * 128],
                                rhs[j][:, :, qlo : qlo + 512],
                                start=(term == 0 and j == 0),
                                stop=(term == 2 and j == DP - 1),
                                perf_mode=DR,
                            )
                for d1t in range(8):
                    dst8 = h8[d1t // 2][:, d1t % 2, qc * 512 : (qc + 1) * 512]
                    dstr = hr8[d1t // 2][:, d1t % 2, qc * 512 : (qc + 1) * 512]
                    nc.scalar.activation(
                        out=dst8,
                        in_=a_ps[d1t],
                        func=mybir.ActivationFunctionType.Copy,
                    )
                    nc.vector.scalar_tensor_tensor(
                        out=dstr,
                        in0=a_ps[d1t],
                        scalar=0.0,
                        in1=dst8,
                        op0=mybir.AluOpType.bypass,
                        op1=mybir.AluOpType.subtract,
                    )

            # ---- Phase B: S = x H; Etil = exp(S/2048) - 1 -> fp8; rowsums --
            for qc in range(2):
                for kt in range(16):
                    sp = pp.tile([128, 512], F32, tag="ps", name=f"sps{sfx}_{kt}_{qc}")
                    for term in range(2):
                        hsrc = h8 if term == 0 else hr8
                        for j in range(DP):
                            nc.tensor.matmul(
                                sp,
                                x8t[j][:, :, kt * 128 : (kt + 1) * 128],
                                hsrc[j][:, :, qc * 512 : (qc + 1) * 512],
                                start=(term == 0 and j == 0),
                                stop=(term == 1 and j == DP - 1),
                                perf_mode=DR,
                            )
                    # E = exp(s) in f32 (Act), then Etil = E - 1 -> fp8 (Pool);
                    # rowsum reduces the exact f32 E (Pool) into rowacc (DVE).
                    ef32 = sb.tile(
                        [128, 512], F32, tag=f"ef32{qc}", bufs=3,
                        name=f"ef32{sfx}_{kt}_{qc}",
                    )
                    nc.scalar.activation(
                        out=ef32,
                        in_=sp,
                        func=mybir.ActivationFunctionType.Exp,
                        scale=EXP_SCALE,
                    )
                    edst = et8[kt // 2][:, kt % 2, qc * 512 : (qc + 1) * 512]
                    nc.gpsimd.tensor_scalar_sub(edst, ef32, 1.0)
                    rtmp = sb.tile(
                        [128, 512], F32, tag=f"rtmp{qc}", bufs=2,
                        name=f"rtmp{sfx}_{kt}_{qc}",
                    )
                    nc.gpsimd.partition_all_reduce(
                        rtmp, ef32, 128, bass_isa.ReduceOp.add
                    )
                    racc = rowacc[:, qc * 512 : (qc + 1) * 512]
                    if kt == 0:
                        nc.vector.tensor_copy(out=racc, in_=rtmp)
                    else:
                        nc.vector.scalar_tensor_tensor(
                            out=racc, in0=rtmp, scalar=0.0, in1=racc,
                            op0=mybir.AluOpType.bypass,
                            op1=mybir.AluOpType.add,
                        )

            # recip2 = 2 / (64 * rowsum): the 2 un-scales c8 (stored as C/2),
            # the 64 un-scales wv (stored as 64 Wv^T).
            filler(4)
            for qc in range(2):
                nc.vector.reciprocal(
                    out=recip2[:, qc * 512 : (qc + 1) * 512],
                    in_=rowacc[:, qc * 512 : (qc + 1) * 512],
                )
            nc.vector.tensor_scalar_mul(recip2, recip2, 1.0 / 32.0)

            # ---- Phase C: Ct = Etil^T x (1 term), colsum chains woven in ---
            # colsum chain dt: [128,1] psum over all 2048 keys of x8n + r8n.
            csp = {}

            def colsum_chain(dt):
                csp[dt] = rs.tile([128, 512], F32, tag="rs", name=f"csp{sfx}_{dt}")[:, 0:1]
                for src in (x8n, r8n):
                    for m in range(KP):
                        nc.tensor.matmul(
                            csp[dt],
                            src[m][:, :, dt * 128 : (dt + 1) * 128],
                            ones8,
                            start=(src is x8n and m == 0),
                            stop=(src is r8n and m == KP - 1),
                            perf_mode=DR,
                        )

            def colsum_evict(dt):
                c_hi = cs8[:, dt % 2, dt // 2 : dt // 2 + 1]
                nc.vector.tensor_scalar_mul(c_hi, csp[dt], 0.125)
                nc.vector.scalar_tensor_tensor(
                    out=csr8[:, dt % 2, dt // 2 : dt // 2 + 1],
                    in0=csp[dt], scalar=0.125, in1=c_hi,
                    op0=mybir.AluOpType.mult,
                    op1=mybir.AluOpType.subtract,
                )

            for ch in range(16):
                dt, qc = divmod(ch, 2)
                c_ps = pp.tile([128, 512], F32, tag="ps", name=f"cps{sfx}_{ch}")
                for m in range(KP):
                    nc.tensor.matmul(
                        c_ps,
                        x8n[m][:, :, dt * 128 : (dt + 1) * 128],
                        et8[m][:, :, qc * 512 : (qc + 1) * 512],
                        start=(m == 0),
                        stop=(m == KP - 1),
                        perf_mode=DR,
                    )
                if ch % 2 == 0:
                    colsum_chain(ch // 2)
                    colsum_evict(ch // 2)
                cdst8 = c8[dt // 2][:, dt % 2, qc * 512 : (qc + 1) * 512]
                nc.scalar.activation(
                    out=cdst8,
                    in_=c_ps,
                    func=mybir.ActivationFunctionType.Copy,
                    scale=0.5,
                )
                nc.vector.scalar_tensor_tensor(
                    out=cr8[dt // 2][:, dt % 2, qc * 512 : (qc + 1) * 512],
                    in0=c_ps, scalar=0.5, in1=cdst8,
                    op0=mybir.AluOpType.mult,
                    op1=mybir.AluOpType.subtract,
                )

            # ---- Phase D: o = (64 Wv)^T C + v0, normalized at eviction ----
            # v0 chain ot: [128,1] psum = (64 Wv)^T (colsum/8); v0sb = x4
            # so the D-evict STT sees v0/2 on the same scale as d_ps = o64/2.
            def v0_chain(ot):
                vp = rs.tile([128, 512], F32, tag="rs", name=f"vp{sfx}_{ot}")[:, 0:1]
                terms = [(wv8, cs8), (wvr8, cs8), (wv8, csr8)]
                for t, (wsrc, csrc) in enumerate(terms):
                    for j in range(DP):
                        nc.tensor.matmul(
                            vp,
                            wsrc[j][:, :, ot * 128 : (ot + 1) * 128],
                            csrc[:, :, j : j + 1],
                            start=(t == 0 and j == 0),
                            stop=(t == 2 and j == DP - 1),
                            perf_mode=DR,
                        )
                nc.vector.tensor_scalar_mul(v0sb[:, ot : ot + 1], vp, 4.0)

            filler(2)
            v0_chain(0)
            v0_chain(1)

            def d_chain(ot, qc, psname, col0, ncol):
                d_ps = pp.tile([128, 512], F32, tag="ps", name=psname)[:, 0:ncol]
                terms = [(wv8, c8), (wvr8, c8), (wv8, cr8)]
                for t, (wsrc, csrc) in enumerate(terms):
                    for j in range(DP):
                        nc.tensor.matmul(
                            d_ps,
                            wsrc[j][:, :, ot * 128 : (ot + 1) * 128],
                            csrc[j][:, :, col0 : col0 + ncol],
                            start=(t == 0 and j == 0),
                            stop=(t == 2 and j == DP - 1),
                            perf_mode=DR,
                        )
                oev = sb.tile(
                    [128, ncol], F32, tag=f"oev{ncol}", bufs=3,
                    name=f"oev{sfx}_{ot}_{col0}",
                )
                nc.vector.scalar_tensor_tensor(
                    out=oev,
                    in0=d_ps,
                    scalar=v0sb[:, ot : ot + 1],
                    in1=recip2[:, col0 : col0 + ncol],
                    op0=mybir.AluOpType.add,
                    op1=mybir.AluOpType.mult,
                )
                dma_eng = nc.sync if (ot + qc) % 2 == 0 else nc.gpsimd
                dma_eng.dma_start(
                    out=out_d[ot * 128 : (ot + 1) * 128, col0 : col0 + ncol],
                    in_=oev,
                )

            for ch in range(15):
                ot, qc = divmod(ch, 2)
                d_chain(ot, qc, f"dps{sfx}_{ch}", qc * 512, 512)
                if 2 <= ch < 8:
                    v0_chain(ch)
            # Final (ot7, qc1) chain as 4 narrow sub-chains: small tail.
            for c4 in range(4):
                d_chain(7, 1, f"fps{sfx}_{c4}", 512 + c4 * 128, 128)
    return nc


def _get_program():
    if "nc" not in _CACHE:
        nc = bacc.Bacc("TRN2", target_bir_lowering=False, num_devices=N_CORES)
        _emit(nc)
        nc.compile()
        _CACHE["nc"] = nc
    return _CACHE["nc"]


def _split8(a):
    f8 = ml_dtypes.float8_e4m3
    hi = a.astype(f8)
    lo = (a - hi.astype(np.float32)).astype(f8)
    return hi, lo


def _pair_t(a):
    """[R, C] -> [R/256, 128, 2, C]: partition-dim tile pairs for DoubleRow."""
    r, c = a.shape
    return np.ascontiguousarray(a.reshape(r // 256, 2, 128, c).transpose(0, 2, 1, 3))


def kernel(x, Wq, Wk, Wv):
    x = np.asarray(x, dtype=np.float32)
    Wq = np.asarray(Wq, dtype=np.float32)
    Wk = np.asarray(Wk, dtype=np.float32)
    Wv = np.asarray(Wv, dtype=np.float32)

    nc = _get_program()
    m8, mr8 = _split8(64.0 * (Wq.T @ Wk))       # [d2, d1], 64x scale
    wv8, wvr8 = _split8(64.0 * Wv.T)            # [d, o], 64x scale
    m8p, mr8p = _pair_t(m8), _pair_t(mr8)
    wv8p, wvr8p = _pair_t(wv8), _pair_t(wvr8)
    in_maps = []
    for c in range(N_CORES):
        b, h = divmod(c, 2)
        xp = np.concatenate(
            [x[b, h * HQ : (h + 1) * HQ], x[b, (1 - h) * HQ : (2 - h) * HQ]], axis=0
        )
        x8, r8 = _split8(xp)                    # [k, d]
        x8t = np.ascontiguousarray(x8.astype(np.float32).T).astype(ml_dtypes.float8_e4m3)
        r8t = np.ascontiguousarray(r8.astype(np.float32).T).astype(ml_dtypes.float8_e4m3)
        in_maps.append(
            {
                "m8": m8p, "mr8": mr8p,
                "x8t": _pair_t(x8t),
                "r8tq": _pair_t(r8t[:, 0:HQ]),
                "x8n": _pair_t(x8), "r8n": _pair_t(r8),
                "wv8": wv8p, "wvr8": wvr8p,
            }
        )
    res = run_bass_kernel_spmd(nc, in_maps, list(range(N_CORES)))
    outp = np.empty((B, S, O), dtype=np.float32)
    for c in range(N_CORES):
        b, h = divmod(c, 2)
        outp[b, h * HQ : (h + 1) * HQ] = res.results[c]["outT"].T
    return outp


# revision 40
# speedup vs baseline: 1.7640x; 1.2706x over previous
"""Trainium2 Bass kernel for single-head attention (no mask), fp8 DoubleRow.

Reference computation (B=4, S=2048, D=1024):
    q = x @ Wq.T ; k = x @ Wk.T ; v = x @ Wv.T          (per batch)
    out = softmax((q @ k.T) / sqrt(1024)) @ v

Sharding: 8 cores = (batch, query-half), same as the bf16 baseline; no
collectives.  Algebra: scores = x (Wq^T Wk) x^T with M = Wq^T Wk
host-prepped, out = softmax(scores) x Wv^T.

All four dense stages run as fp8e4 (e4m3) DoubleRow matmuls: the PE
contracts two 128-row k-tiles per instruction at 0.5 cycles/moving-row,
4x the bf16 MAC rate.  e4m3's ~3.6% quantization noise is managed by
hi+lo residual splitting (a = fp8(a) + fp8(a - fp8(a))) with the number
of product terms chosen per stage, and by a Taylor shift of the softmax:
    E = exp(s) = 1 + Etil,   C = E^T x = colsum(x) (+) Etil^T x
so the rank-1 mass of E (the dominant part) flows through an exact f32
side-channel (colsum via tiny ones-matmuls on the PE, ~free) and only
the small Etil (std ~0.37) is quantized -- cutting its error ~3x.

Stages (per core, q = the core's 1024 queries, 64x scale keeps fp8
operands out of the denormal range):
    A: H = (64 M)^T xq^T        [d,q]  terms m8*x8 + mr8*x8 + m8*r8
    B: S = x H                  [k,q]  terms x8*(h8 + hr8)
    Etil = exp(S/2048) - 1 - fp8 (Act bias), rowsum via Pool reduce
    C: Ct = Etil^T x            [d,q]  term  x8^T e8
    D: o = (64 Wv^T)^T C        [o,q]  terms (wv8+wvr8)*c8 + wv8*cr8
       + v0 = (64 Wv)^T colsum  [o,1]  via tiny [*,1] DoubleRow chains
    evict: out = (d_ps + v0/2) * (2/(64*rowsum))   (one DVE STT)

Error (vs f32 reference, measured): 1.58e-2 mean-rel (gate 2e-2).
PE work: (49.2 + 65.5 + 32.8 + 49.2)k cycles ~= 82 us @ 2.4 GHz vs
393k cycles (164 us) for the bf16 baseline.

Scheduling: one in-order SP DMA queue ordered by first use; a warmup
matmul chain bridges the initial DMA latency; phase-A waves are
qc-major so phase B can chase wave-0's evictions; filler matmuls at
phase junctions keep the PE stream gap-free (the cost model drops to
the mid p-state for 3 us after any idle gap); tiny colsum/v0 chains
interleave into the C/D instruction stream using a reserved PSUM bank;
the final output chain is split into narrow [128,128] sub-chains so the
tail is one small evict+DMA.
"""

import ml_dtypes
import numpy as np

import concourse.bass as bass
import concourse.tile as tile
from concourse import bacc, bass_isa, mybir
from concourse.bass_utils import run_bass_kernel_spmd

B, S, D, O = 4, 2048, 1024, 1024
HQ = S // 2  # query rows per core
N_CORES = 8
BF = mybir.dt.bfloat16
F8 = mybir.dt.float8e4
F32 = mybir.dt.float32
DR = mybir.MatmulPerfMode.DoubleRow
EXP_SCALE = 1.0 / (32.0 * 64.0)  # softmax 1/sqrt(1024) and the 64x M scale
DP = D // 256  # 4 contraction pair-tiles over d
KP = S // 256  # 8 key pair-tiles
NWARM = 22  # warmup matmuls bridging the initial DMA latency

_CACHE: dict = {}


def _emit(nc, sfx=""):
    m8_d = nc.dram_tensor(f"m8{sfx}", [DP, 128, 2, D], F8, kind="ExternalInput")
    mr8_d = nc.dram_tensor(f"mr8{sfx}", [DP, 128, 2, D], F8, kind="ExternalInput")
    x8t_d = nc.dram_tensor(f"x8t{sfx}", [DP, 128, 2, S], F8, kind="ExternalInput")
    r8tq_d = nc.dram_tensor(f"r8tq{sfx}", [DP, 128, 2, HQ], F8, kind="ExternalInput")
    x8n_d = nc.dram_tensor(f"x8n{sfx}", [KP, 128, 2, D], F8, kind="ExternalInput")
    r8n_d = nc.dram_tensor(f"r8n{sfx}", [KP, 128, 2, D], F8, kind="ExternalInput")
    wv8_d = nc.dram_tensor(f"wv8{sfx}", [DP, 128, 2, O], F8, kind="ExternalInput")
    wvr8_d = nc.dram_tensor(f"wvr8{sfx}", [DP, 128, 2, O], F8, kind="ExternalInput")
    out_d = nc.dram_tensor(f"outT{sfx}", [O, HQ], BF, kind="ExternalOutput")

    with tile.TileContext(nc) as tc:
        with (
            tc.tile_pool(name=f"{sfx}sb", bufs=1) as sb,
            tc.tile_pool(name=f"{sfx}pp", bufs=7, space="PSUM") as pp,
            tc.tile_pool(name=f"{sfx}rs", bufs=1, space="PSUM") as rs,
        ):
            m8 = [sb.tile([128, 2, D], F8, tag=f"m8_{j}", name=f"m8{sfx}_{j}") for j in range(DP)]
            mr8 = [sb.tile([128, 2, D], F8, tag=f"mr8_{j}", name=f"mr8{sfx}_{j}") for j in range(DP)]
            x8t = [sb.tile([128, 2, S], F8, tag=f"x8t_{j}", name=f"x8t{sfx}_{j}") for j in range(DP)]
            r8tq = [sb.tile([128, 2, HQ], F8, tag=f"r8tq_{j}", name=f"r8tq{sfx}_{j}") for j in range(DP)]
            x8n = [sb.tile([128, 2, D], F8, tag=f"x8n_{m}", name=f"x8n{sfx}_{m}") for m in range(KP)]
            r8n = [sb.tile([128, 2, D], F8, tag=f"r8n_{m}", name=f"r8n{sfx}_{m}") for m in range(KP)]
            wv8 = [sb.tile([128, 2, O], F8, tag=f"wv8_{j}", name=f"wv8{sfx}_{j}") for j in range(DP)]
            wvr8 = [sb.tile([128, 2, O], F8, tag=f"wvr8_{j}", name=f"wvr8{sfx}_{j}") for j in range(DP)]
            h8 = [sb.tile([128, 2, HQ], F8, tag=f"h8_{j}", name=f"h8{sfx}_{j}") for j in range(DP)]
            hr8 = [sb.tile([128, 2, HQ], F8, tag=f"hr8_{j}", name=f"hr8{sfx}_{j}") for j in range(DP)]
            et8 = [sb.tile([128, 2, HQ], F8, tag=f"et8_{m}", name=f"et8{sfx}_{m}") for m in range(KP)]
            c8 = [sb.tile([128, 2, HQ], F8, tag=f"c8_{j}", name=f"c8{sfx}_{j}") for j in range(DP)]
            cr8 = [sb.tile([128, 2, HQ], F8, tag=f"cr8_{j}", name=f"cr8{sfx}_{j}") for j in range(DP)]
            cs8 = sb.tile([128, 2, DP], F8, tag="cs8", name=f"cs8{sfx}")
            csr8 = sb.tile([128, 2, DP], F8, tag="csr8", name=f"csr8{sfx}")
            # 32 columns: a 2-column fp8 stationary trips walrus's
            # s3_lw_dual_fp8_restrictions ISA check in the rowsum chains.
            ones8 = sb.tile([128, 2, 32], F8, tag="ones8", name=f"ones8{sfx}")
            v0sb = sb.tile([128, 8], F32, tag="v0sb", name=f"v0sb{sfx}")
            warmt = sb.tile([128, 512], BF, tag="warm", name=f"warmt{sfx}")
            rsum = sb.tile([128, HQ], F32, tag="rsum", name=f"rsum{sfx}")
            recip2 = sb.tile([128, HQ], F32, tag="recip2", name=f"recip2{sfx}")

            # Constants via memset (no DMA bandwidth). warmt on DVE so the
            # warmup chain can start early; ones8 (fp8) on Pool.
            nc.gpsimd.memset(warmt, 0.0)
            nc.gpsimd.memset(ones8, 1.0)
            # Dummy exp: hoists the Act engine's one-time activation-table
            # load into the idle startup window.
            actwarm = sb.tile([128, 1], BF, tag="actwarm", name=f"actwarm{sfx}")
            nc.scalar.activation(
                out=actwarm,
                in_=warmt[:, 0:1],
                func=mybir.ActivationFunctionType.Exp,
                scale=EXP_SCALE,
            )

            # ---- DMA loads: one in-order queue = explicit priority ----
            # x8t is split so phase A's critical set (m8 + query columns)
            # lands first; the key columns follow before phase B needs them.
            for j in range(DP):
                nc.sync.dma_start(out=m8[j], in_=m8_d[j])
                nc.sync.dma_start(out=x8t[j][:, :, 0:HQ], in_=x8t_d[j][:, :, 0:HQ])
            for j in range(DP):
                nc.sync.dma_start(out=mr8[j], in_=mr8_d[j])
            for j in range(DP):
                nc.sync.dma_start(out=r8tq[j], in_=r8tq_d[j])
            for j in range(DP):
                nc.sync.dma_start(out=x8t[j][:, :, HQ:S], in_=x8t_d[j][:, :, HQ:S])
            for m in range(KP):
                nc.sync.dma_start(out=x8n[m], in_=x8n_d[m])
            for m in range(KP):
                nc.sync.dma_start(out=r8n[m], in_=r8n_d[m])
            for j in range(DP):
                nc.sync.dma_start(out=wv8[j], in_=wv8_d[j])
            for j in range(DP):
                nc.sync.dma_start(out=wvr8[j], in_=wvr8_d[j])

            # ---- PE warmup: hold the p-state until the first loads land ----
            wps = rs.tile([128, 512], F32, tag="rs", name=f"wps{sfx}")
            for i in range(NWARM):
                nc.tensor.matmul(
                    wps[:, 0:128], warmt[:, 0:128], warmt[:, 0:128],
                    start=True, stop=True,
                )

            fill_n = [0]

            def filler(n):
                # Junction filler: keeps the PE busy (p-state) while vector
                # engines produce the next phase's operands.
                fill_n[0] += 1
                fps = rs.tile([128, 512], F32, tag="rs", name=f"fil{sfx}_{fill_n[0]}")
                for _ in range(n):
                    nc.tensor.matmul(
                        fps, warmt[:, 0:128], warmt, start=True, stop=True
                    )

            # ---- Phase A: H = (64 M)^T xq^T, evicted as fp8 hi+lo ----
            # Wave 0 = qc 0 (so phase B's qc-0 chains can chase wave-0's
            # evictions), wave 1 = qc 1.  Terms j-major so the DMA feed
            # (m8, x8t first, then mr8, then r8tq) is consumed in order.
            for qc in range(2):
                a_ps = {}
                for d1t in range(8):
                    pool = pp if d1t < 7 else rs
                    tag = "ps" if d1t < 7 else "rs"
                    a_ps[d1t] = pool.tile([128, 512], F32, tag=tag, name=f"aps{sfx}_{qc}_{d1t}")
                qlo = qc * 512
                for term in range(2):
                    lhs, rhs = [(m8, x8t), (mr8, x8t)][term]
                    for j in range(DP):
                        for d1t in range(8):
                            nc.tensor.matmul(
                                a_ps[d1t],
                                lhs[j][:, :, d1t * 128 : (d1t + 1) * 128],
                                rhs[j][:, :, qlo : qlo + 512],
                                start=(term == 0 and j == 0),
                                stop=False,
                                perf_mode=DR,
                            )
                # Final term chain-major with inline evictions so chains
                # finish staggered and the eviction burst overlaps the wave.
                for d1t in range(8):
                    for j in range(DP):
                        nc.tensor.matmul(
                            a_ps[d1t],
                            m8[j][:, :, d1t * 128 : (d1t + 1) * 128],
                            r8tq[j][:, :, qlo : qlo + 512],
                            start=False,
                            stop=(j == DP - 1),
                            perf_mode=DR,
                        )
                    dst8 = h8[d1t // 2][:, d1t % 2, qc * 512 : (qc + 1) * 512]
                    dstr = hr8[d1t // 2][:, d1t % 2, qc * 512 : (qc + 1) * 512]
                    nc.scalar.activation(
                        out=dst8,
                        in_=a_ps[d1t],
                        func=mybir.ActivationFunctionType.Copy,
                    )
                    nc.vector.scalar_tensor_tensor(
                        out=dstr,
                        in0=a_ps[d1t],
                        scalar=0.0,
                        in1=dst8,
                        op0=mybir.AluOpType.bypass,
                        op1=mybir.AluOpType.subtract,
                    )

            # ---- Phase B: S = x H; Etil = exp(S/2048) - 1 -> fp8; rowsums --
            for qc in range(2):
                for kt in range(16):
                    sp = pp.tile([128, 512], F32, tag="ps", name=f"sps{sfx}_{kt}_{qc}")
                    for term in range(2):
                        hsrc = h8 if term == 0 else hr8
                        for j in range(DP):
                            nc.tensor.matmul(
                                sp,
                                x8t[j][:, :, kt * 128 : (kt + 1) * 128],
                                hsrc[j][:, :, qc * 512 : (qc + 1) * 512],
                                start=(term == 0 and j == 0),
                                stop=(term == 1 and j == DP - 1),
                                perf_mode=DR,
                            )
                    # E = exp(s) in f32 (Act), then Etil = E - 1 -> fp8 (Pool);
                    # rowsum reduces the exact f32 E (Pool) into rowacc (DVE).
                    ef32 = sb.tile(
                        [128, 512], F32, tag=f"ef32{qc}", bufs=3,
                        name=f"ef32{sfx}_{kt}_{qc}",
                    )
                    nc.scalar.activation(
                        out=ef32,
                        in_=sp,
                        func=mybir.ActivationFunctionType.Exp,
                        scale=EXP_SCALE,
                    )
                    # Etil = E - 1 -> fp8 (DVE/Pool alternate; Act only does
                    # the exp, so no engine saturates the 853ns chain period).
                    edst = et8[kt // 2][:, kt % 2, qc * 512 : (qc + 1) * 512]
                    if kt % 2 == 0:
                        nc.vector.tensor_scalar_sub(edst, ef32, 1.0)
                    else:
                        nc.gpsimd.tensor_scalar_sub(edst, ef32, 1.0)

            filler(8)

            # ---- Phase C: Ct = Etil^T x (1 term), colsum chains woven in ---
            # colsum chain dt: [128,1] psum over all 2048 keys of x8n + r8n.
            csp = {}

            def colsum_chain(dt):
                csp[dt] = rs.tile([128, 512], F32, tag="rs", name=f"csp{sfx}_{dt}")[:, 0:1]
                for src in (x8n, r8n):
                    for m in range(KP):
                        nc.tensor.matmul(
                            csp[dt],
                            src[m][:, :, dt * 128 : (dt + 1) * 128],
                            ones8[:, :, 0:1],
                            start=(src is x8n and m == 0),
                            stop=(src is r8n and m == KP - 1),
                            perf_mode=DR,
                        )

            def colsum_evict(dt):
                c_hi = cs8[:, dt % 2, dt // 2 : dt // 2 + 1]
                nc.scalar.activation(
                    out=c_hi, in_=csp[dt],
                    func=mybir.ActivationFunctionType.Copy, scale=0.125,
                )
                nc.vector.scalar_tensor_tensor(
                    out=csr8[:, dt % 2, dt // 2 : dt // 2 + 1],
                    in0=csp[dt], scalar=0.125, in1=c_hi,
                    op0=mybir.AluOpType.mult,
                    op1=mybir.AluOpType.subtract,
                )

            # Softmax denominators on the PE: rowsum_q = sum_k Etil[k,q] via
            # ones-stationary chains ([1,512] psum each), then
            # recip2 = 2 / (64 * (2048 + rowsum)) broadcast to all partitions
            # (the 2 un-scales c8 = C/2, the 64 un-scales wv = 64 Wv^T).
            for qc in range(2):
                r_ps = pp.tile([128, 512], F32, tag="ps", name=f"rps{sfx}_{qc}")[0:32, :]
                for m in range(KP):
                    nc.tensor.matmul(
                        r_ps,
                        ones8,
                        et8[m][:, :, qc * 512 : (qc + 1) * 512],
                        start=(m == 0),
                        stop=(m == KP - 1),
                        perf_mode=DR,
                    )
                nc.vector.tensor_scalar_add(
                    rsum[0:1, qc * 512 : (qc + 1) * 512], r_ps[0:1, :], 2048.0
                )
            rb = rsum[0:1, :]
            nc.vector.reciprocal(out=rb, in_=rb)
            nc.vector.tensor_scalar_mul(rb, rb, 1.0 / 32.0)
            nc.gpsimd.partition_broadcast(recip2, rb, 128)

            for ch in range(16):
                dt, qc = divmod(ch, 2)
                c_ps = pp.tile([128, 512], F32, tag="ps", name=f"cps{sfx}_{ch}")
                for m in range(KP):
                    nc.tensor.matmul(
                        c_ps,
                        x8n[m][:, :, dt * 128 : (dt + 1) * 128],
                        et8[m][:, :, qc * 512 : (qc + 1) * 512],
                        start=(m == 0),
                        stop=(m == KP - 1),
                        perf_mode=DR,
                    )
                if ch % 2 == 0:
                    colsum_chain(ch // 2)
                    colsum_evict(ch // 2)
                cdst8 = c8[dt // 2][:, dt % 2, qc * 512 : (qc + 1) * 512]
                nc.scalar.activation(
                    out=cdst8,
                    in_=c_ps,
                    func=mybir.ActivationFunctionType.Copy,
                    scale=0.5,
                )
                nc.vector.scalar_tensor_tensor(
                    out=cr8[dt // 2][:, dt % 2, qc * 512 : (qc + 1) * 512],
                    in0=c_ps, scalar=0.5, in1=cdst8,
                    op0=mybir.AluOpType.mult,
                    op1=mybir.AluOpType.subtract,
                )

            # ---- Phase D: o = (64 Wv)^T C + v0, normalized at eviction ----
            # v0 chain ot: [128,1] psum = (64 Wv)^T (colsum/8); v0sb = x4
            # so the D-evict STT sees v0/2 on the same scale as d_ps = o64/2.
            def v0_chain(ot):
                vp = rs.tile([128, 512], F32, tag="rs", name=f"vp{sfx}_{ot}")[:, 0:1]
                terms = [(wv8, cs8), (wvr8, cs8), (wv8, csr8)]
                for t, (wsrc, csrc) in enumerate(terms):
                    for j in range(DP):
                        nc.tensor.matmul(
                            vp,
                            wsrc[j][:, :, ot * 128 : (ot + 1) * 128],
                            csrc[:, :, j : j + 1],
                            start=(t == 0 and j == 0),
                            stop=(t == 2 and j == DP - 1),
                            perf_mode=DR,
                        )
                nc.vector.tensor_scalar_mul(v0sb[:, ot : ot + 1], vp, 4.0)

            filler(5)
            v0_chain(0)
            v0_chain(1)

            def d_chain(ot, qc, col0, ncol):
                d_ps = pp.tile(
                    [128, 512], F32, tag="ps", name=f"dps{sfx}_{ot}_{col0}"
                )[:, 0:ncol]
                terms = [(wv8, c8), (wvr8, c8), (wv8, cr8)]
                for t, (wsrc, csrc) in enumerate(terms):
                    for j in range(DP):
                        nc.tensor.matmul(
                            d_ps,
                            wsrc[j][:, :, ot * 128 : (ot + 1) * 128],
                            csrc[j][:, :, col0 : col0 + ncol],
                            start=(t == 0 and j == 0),
                            stop=(t == 2 and j == DP - 1),
                            perf_mode=DR,
                        )
                oev = sb.tile(
                    [128, ncol], BF, tag=f"oev{ncol}", bufs=4,
                    name=f"oev{sfx}_{ot}_{col0}",
                )
                nc.vector.scalar_tensor_tensor(
                    out=oev,
                    in0=d_ps,
                    scalar=v0sb[:, ot : ot + 1],
                    in1=recip2[:, col0 : col0 + ncol],
                    op0=mybir.AluOpType.add,
                    op1=mybir.AluOpType.mult,
                )
                # SP queue only: gpsimd DMA takes the slow SWDGE path (~1us
                # Pool-side descriptor gen) which bloats the tail.
                nc.sync.dma_start(
                    out=out_d[ot * 128 : (ot + 1) * 128, col0 : col0 + ncol],
                    in_=oev,
                )

            for ch in range(15):
                ot, qc = divmod(ch, 2)
                d_chain(ot, qc, qc * 512, 512)
                if 2 <= ch < 8:
                    v0_chain(ch)
            # Final chain as two pipelined halves: the first half's eviction
            # and store descriptor-gen overlap the second half's matmuls.
            d_chain(7, 1, 512, 256)
            d_chain(7, 1, 768, 256)
    return nc


def _get_program():
    if "nc" not in _CACHE:
        nc = bacc.Bacc("TRN2", target_bir_lowering=False, num_devices=N_CORES)
        _emit(nc)
        nc.compile()
        _CACHE["nc"] = nc
    return _CACHE["nc"]


def _split8(a):
    f8 = ml_dtypes.float8_e4m3
    hi = a.astype(f8)
    lo = (a - hi.astype(np.float32)).astype(f8)
    return hi, lo


def _pair_t(a):
    """[R, C] -> [R/256, 128, 2, C]: partition-dim tile pairs for DoubleRow."""
    r, c = a.shape
    return np.ascontiguousarray(a.reshape(r // 256, 2, 128, c).transpose(0, 2, 1, 3))


def kernel(x, Wq, Wk, Wv):
    x = np.asarray(x, dtype=np.float32)
    Wq = np.asarray(Wq, dtype=np.float32)
    Wk = np.asarray(Wk, dtype=np.float32)
    Wv = np.asarray(Wv, dtype=np.float32)

    nc = _get_program()
    m8, mr8 = _split8(64.0 * (Wq.T @ Wk))       # [d2, d1], 64x scale
    wv8, wvr8 = _split8(64.0 * Wv.T)            # [d, o], 64x scale
    m8p, mr8p = _pair_t(m8), _pair_t(mr8)
    wv8p, wvr8p = _pair_t(wv8), _pair_t(wvr8)
    in_maps = []
    for c in range(N_CORES):
        b, h = divmod(c, 2)
        xp = np.concatenate(
            [x[b, h * HQ : (h + 1) * HQ], x[b, (1 - h) * HQ : (2 - h) * HQ]], axis=0
        )
        x8, r8 = _split8(xp)                    # [k, d]
        x8t = np.ascontiguousarray(x8.astype(np.float32).T).astype(ml_dtypes.float8_e4m3)
        r8t = np.ascontiguousarray(r8.astype(np.float32).T).astype(ml_dtypes.float8_e4m3)
        in_maps.append(
            {
                "m8": m8p, "mr8": mr8p,
                "x8t": _pair_t(x8t),
                "r8tq": _pair_t(r8t[:, 0:HQ]),
                "x8n": _pair_t(x8), "r8n": _pair_t(r8),
                "wv8": wv8p, "wvr8": wvr8p,
            }
        )
    res = run_bass_kernel_spmd(nc, in_maps, list(range(N_CORES)))
    outp = np.empty((B, S, O), dtype=np.float32)
    for c in range(N_CORES):
        b, h = divmod(c, 2)
        outp[b, h * HQ : (h + 1) * HQ] = res.results[c]["outT"].astype(np.float32).T
    return outp
